# revision 26
# baseline (speedup 1.0000x reference)
"""Multi-head attention (per-head projections + relative position bias) on 8
Trainium2 NeuronCores.

Sharding: core c -> batch c//4, heads 4*(c%4) .. 4*(c%4)+4 (tensor parallel
over heads within a batch). Each core computes its 4 heads end-to-end plus the
partial output projection for those heads; the host sums the 4 partials per
batch and adds bfc.

Device-side design:
- bias folded multiplicatively: host ships expb = exp(bias/8) fp16; ACT does
  es0 = exp(scores/8) directly from PSUM (its fast port); DVE multiplies
  es = es0*expb in fp16 SBUF at 2x mode (instead of a 1x-mode fp32 PSUM add).
- one shared 4-bank PSUM work ring + 2x2-bank AV accumulators; projections, V,
  scores and FC all interleave through it so the PE never idles long enough to
  re-throttle the HAM clock gate.
- AV emission lags scores by 2 tiles so the in-order PE queue doesn't stall on
  exp latency.
- q/k biases preloaded into PSUM by K=1 ones-product matmuls; evacuations are
  plain DVE copies.
- softmax denominators ride the V ones-column (65th AV output row);
  reciprocal_approx_fast + a K=1 broadcast matmul builds the normalizer.
- FC contracts K=128 (head pair stacked on partitions via partition-shifted
  normalize writes); qh=0 FC interleaves into group 2, qh=1 FC is the tail.
- fp16 output partials; x loaded in halves so compute starts ~15us in.
"""

import sys

sys.path.insert(0, "/opt/trn_rl_repo")

import numpy as np

import concourse.bass as bass
import concourse.tile as tile_mod
from concourse import mybir

# ---------------------------------------------------------------------------
# This walrus build accepts only one sem-wait per CTRL/Drain instruction, so
# split the TileContext tail drain's waits onto individual single-wait nops.
# ---------------------------------------------------------------------------


def _patched_drain_and_barrier(self, tick_clock, wait_clock):
    nc = self.nc
    drain_inst = nc.sync.drain()
    wait_clock.add_sem_waits(
        drain_inst.ins, tile_mod.ScopedClock({None: tick_clock.global_clock})
    )
    si = drain_inst.ins.sync_info
    if si is not None and si.on_wait is not None and len(si.on_wait) > 1:
        waits = list(si.on_wait)
        si.on_wait = [waits[0]]
        for w in waits[1:]:
            n = nc.sync.nop()
            n.ins.sync_info = mybir.SyncInfo(on_wait=[w], on_update=[])

    nc.all_engine_barrier()
    assert self.sems is not None
    popped = nc._tile_sem_poison_stack.pop()
    assert popped is self._sem_poison
    nc.clear_and_free_semaphores(list(self.sems.allocated().values()))
    nc.all_engine_barrier()


tile_mod.TileContext._drain_and_barrier = _patched_drain_and_barrier

_split_ctr = [0]


def _split_multi_waits(nc):
    """Walrus here accepts a single sem-wait per instruction; hoist extra waits
    onto single-wait nops inserted just before, on the same engine."""
    for f in nc.m.functions:
        for bb in f.blocks:
            insts = bb.instructions
            out = []
            for inst in insts:
                si = inst.sync_info
                if si is not None and si.on_wait is not None and len(si.on_wait) > 1:
                    waits = list(si.on_wait)
                    for w in waits[:-1]:
                        _split_ctr[0] += 1
                        n = mybir.InstNoOp(name=f"splitw-{_split_ctr[0]}", ins=[], outs=[])
                        n.engine = inst.engine
                        n.sync_info = mybir.SyncInfo(on_wait=[w], on_update=[])
                        out.append(n)
                    inst.sync_info = mybir.SyncInfo(
                        on_wait=[waits[-1]], on_update=list(si.on_update or [])
                    )
                out.append(inst)
            if len(out) != len(insts):
                bb.instructions[:] = out


B, S, D, H, DH = 2, 2048, 1024, 16, 64
NCORES = 8
HPC = 4  # heads per core
P = 128
F16 = mybir.dt.float16
F32 = mybir.dt.float32
AF = mybir.ActivationFunctionType
OP = mybir.AluOpType

AV_LAG = 2
NT = 16  # t tiles of 128

_cached = {}


def _build_program(split_waits=True):
    nt, nqh = NT, 2
    Sl = nt * P
    Sq = nqh * 1024
    groups = ((0, 0), (1, 0), (0, 1), (1, 1))  # (pr, qh)

    nc = bass.Bass("TRN2", target_bir_lowering=False, debug=False)

    d_xq = nc.dram_tensor("xq", [P, 8, Sq], F16, kind="ExternalInput").ap()
    d_xk = nc.dram_tensor("xk", [P, 8, Sl], F16, kind="ExternalInput").ap()
    d_xv = nc.dram_tensor("xv", [P, 8, Sl], F16, kind="ExternalInput").ap()
    d_wq = nc.dram_tensor("wq", [P, 2, 8, P], F16, kind="ExternalInput").ap()
    d_wk = nc.dram_tensor("wk", [P, 2, 8, P], F16, kind="ExternalInput").ap()
    d_bqk = nc.dram_tensor("bqk", [P, 2, 2, 1], F32, kind="ExternalInput").ap()
    d_wv = nc.dram_tensor("wv", [P, 8, HPC * 65], F16, kind="ExternalInput").ap()
    # ones-column seed: bv128[k, j] = (j % 65 == 64) / 128, so a K=128 matmul
    # against all-ones weights initializes vv's denominator columns to 1.
    d_bv = nc.dram_tensor("bv", [P, HPC * 65], F16, kind="ExternalInput").ap()
    d_eb = nc.dram_tensor("eb", [2, nqh, nt, P, 2, 1024], F16, kind="ExternalInput").ap()
    d_wfc = nc.dram_tensor("wfc", [P, 2, D], F16, kind="ExternalInput").ap()
    d_sel = nc.dram_tensor("sel", [8, 512], F32, kind="ExternalInput").ap()
    d_out = nc.dram_tensor("out", [Sq, D], F16, kind="ExternalOutput").ap()

    with tile_mod.TileContext(nc) as tc:
        with tc.tile_pool(name="persist", bufs=1) as persist, \
             tc.tile_pool(name="work", bufs=2, space="PSUM") as work, \
             tc.tile_pool(name="ps_av", bufs=1, space="PSUM") as ps_av, \
             tc.tile_pool(name="ebp", bufs=5) as ebp, \
             tc.tile_pool(name="es0p", bufs=3) as es0p, \
             tc.tile_pool(name="es1p", bufs=3) as es1p, \
             tc.tile_pool(name="gep", bufs=2) as gep, \
             tc.tile_pool(name="fop", bufs=2) as fop:
            qT = persist.tile([P, 2, Sq], F16, tag="qT")  # [j2, pr, q]
            kT = persist.tile([P, 2, Sl], F16, tag="kT")  # [j2, pr, t]
            vv = persist.tile([P, nt, HPC * 65], F16, tag="vv")  # [t_in, tt, h*65+j]
            onorm = persist.tile([P, 2, Sq], F16, tag="onorm")  # [j2, pr, q]
            wq_sb = persist.tile([P, 2, 8, P], F16, tag="wq")
            wk_sb = persist.tile([P, 2, 8, P], F16, tag="wk")
            wv_sb = persist.tile([P, 8, HPC * 65], F16, tag="wv")
            wfc_sb = persist.tile([P, 2, D], F16, tag="wfc")
            xq_sb = persist.tile([P, 8, Sq], F16, tag="xq")
            xk_sb = persist.tile([P, 8, Sl], F16, tag="xk")
            xv_sb = persist.tile([P, 8, Sl], F16, tag="xv")
            bqk_sb = persist.tile([P, 2, 2, 1], F32, tag="bqk")
            bv_sb = persist.tile([P, HPC * 65], F16, tag="bv")
            ones_sb = persist.tile([P, P], F16, tag="ones")
            sel_sb = persist.tile([8, 512], F32, tag="sel")

            nc.vector.memset(ones_sb[:], 1.0)
            # --- DMA emission order matters: unblock k(pr0)/q(pr0,qh0) fast ---
            nc.sync.dma_start(sel_sb[:], d_sel)
            nc.sync.dma_start(bqk_sb[:], d_bqk)
            nc.sync.dma_start(bv_sb[:], d_bv)
            nc.sync.dma_start(wq_sb[:], d_wq)
            nc.sync.dma_start(wk_sb[:], d_wk)
            half = Sq // 2
            nc.sync.dma_start(xk_sb[:, :, 0:Sl // 2], d_xk[:, :, 0:Sl // 2])
            nc.sync.dma_start(xq_sb[:, :, 0:half], d_xq[:, :, 0:half])
            nc.sync.dma_start(xk_sb[:, :, Sl // 2:Sl], d_xk[:, :, Sl // 2:Sl])
            nc.sync.dma_start(wv_sb[:], d_wv)
            # xv in per-256 t chunks so V(tt) unblocks early
            for i in range(nt // 2):
                nc.sync.dma_start(
                    xv_sb[:, :, i * 256:(i + 1) * 256], d_xv[:, :, i * 256:(i + 1) * 256]
                )
            nc.sync.dma_start(xq_sb[:, :, half:Sq], d_xq[:, :, half:Sq])
            nc.sync.dma_start(wfc_sb[:], d_wfc)

            def emit_qk_tile(ty, pr, sh):
                """One projection tile -> qT/kT[:, pr, sh*1024:(sh+1)*1024]."""
                w_sb, x_sb, dst = (
                    (wq_sb, xq_sb, qT) if ty == 0 else (wk_sb, xk_sb, kT)
                )
                pw = work.tile([P, 1024], F32, tag="w", name=f"pw{ty}{pr}{sh}")
                for qc in range(2):
                    for dp in range(8):
                        nc.tensor.matmul(
                            pw[:, qc * 512:(qc + 1) * 512],
                            lhsT=w_sb[:, pr, dp, :],
                            rhs=x_sb[:, dp, sh * 1024 + qc * 512: sh * 1024 + (qc + 1) * 512],
                            start=(dp == 0),
                            stop=(dp == 7),
                        )
                nc.vector.tensor_scalar(
                    dst[:, pr, sh * 1024:(sh + 1) * 1024],
                    pw[:],
                    bqk_sb[:, ty, pr, :],
                    None,
                    OP.add,
                )

            def emit_v_tile(tt):
                pv = work.tile([P, 1024], F32, tag="w", name=f"pv{tt}")
                nc.tensor.matmul(
                    pv[:, 0:HPC * 65], lhsT=ones_sb[:], rhs=bv_sb[:],
                    start=True, stop=False,
                )
                for dp in range(8):
                    nc.tensor.matmul(
                        pv[:, 0:HPC * 65],
                        lhsT=xv_sb[:, dp, tt * P:(tt + 1) * P],
                        rhs=wv_sb[:, dp, :],
                        start=False,
                        stop=(dp == 7),
                    )
                nc.vector.tensor_copy(vv[:, tt, :], pv[:, 0:HPC * 65])

            def emit_fc_tile(qt):
                pf = work.tile([P, 1024], F32, tag="w", name=f"pf{qt}")
                for ec in range(2):
                    for i2 in range(2):
                        nc.tensor.matmul(
                            pf[:, ec * 512:(ec + 1) * 512],
                            lhsT=onorm[:, i2, qt * P:(qt + 1) * P],
                            rhs=wfc_sb[:, i2, ec * 512:(ec + 1) * 512],
                            start=(i2 == 0),
                            stop=(i2 == 1),
                        )
                fo = fop.tile([P, D], F16, tag="fo", name=f"fo{qt}")
                nc.vector.tensor_copy(fo[:], pf[:])
                nc.sync.dma_start(d_out[qt * P:(qt + 1) * P, :], fo[:])

            # remaining projection tiles interleaved into group 0's stream
            extra_qk = [(1, 1, 0), (1, 1, 1), (0, 1, 0), (0, 0, 1), (0, 1, 1)]
            qk_slots = {
                max(1, (i + 1) * nt // (len(extra_qk) + 1)): t
                for i, t in enumerate(extra_qk)
            }
            assert len(qk_slots) == len(extra_qk)

            # phase A head: what group 0 needs first (k01 last — its x half
            # arrives after xq's first half)
            emit_qk_tile(1, 0, 0)
            emit_qk_tile(0, 0, 0)
            emit_qk_tile(1, 0, 1)

            # normalize tails (rb matmul + onorm mult) deferred into the NEXT
            # group's instruction stream so the PE never waits on the
            # reciprocal chain at a group boundary.
            pending_norm = []

            def emit_norm_tail():
                pr0, qh0, hh, oT, rec8 = pending_norm.pop(0)
                rbt = work.tile([P, 1024], F32, tag="w", name=f"rb{pr0}{qh0}{hh}")
                for c in range(8):
                    nc.tensor.matmul(
                        rbt[0:64, c * P:(c + 1) * P],
                        lhsT=sel_sb[:, c * 64:(c + 1) * 64],
                        rhs=rec8[:],
                        start=True,
                        stop=True,
                    )
                nc.vector.tensor_tensor(
                    onorm[hh * 64:(hh + 1) * 64, pr0, qh0 * 1024:(qh0 + 1) * 1024],
                    oT[0:64, :],
                    rbt[0:64, :],
                    OP.mult,
                )

            # AV matmuls pend globally: a group's last AV batches interleave
            # with the NEXT group's score stream so the PE never drains dry
            # at a group boundary.
            pend = []
            po_by_gi = {}

            def emit_av():
                gi0, pr0, tt0, es1_t = pend.pop(0)
                if tt0 == 0:
                    # allocate accumulators only now, AFTER the previous
                    # group's final AV matmuls were emitted on these slots
                    po_by_gi[gi0] = [
                        ps_av.tile([65, 1024], F32, tag=f"po{hh}",
                                   name=f"po{gi0}{hh}")
                        for hh in range(2)
                    ]
                po_g = po_by_gi[gi0]
                for hh in range(2):
                    h = 2 * pr0 + hh
                    for qc in range(2):
                        nc.tensor.matmul(
                            po_g[hh][:, qc * 512:(qc + 1) * 512],
                            lhsT=vv[:, tt0, h * 65:(h + 1) * 65],
                            rhs=es1_t[:, hh, qc * 512:(qc + 1) * 512],
                            start=(tt0 == 0),
                            stop=(tt0 == nt - 1),
                        )
                if tt0 == nt - 1:
                    # group finished: kick off its normalization chain
                    emit_norm_head(po_g, pr0)

            norm_ctr = [0]

            def emit_norm_head(po_g, pr0):
                gi0 = norm_ctr[0]
                norm_ctr[0] += 1
                qh0 = groups[gi0][1]
                for hh in range(2):
                    oT = gep.tile([65, 1024], F32, tag="oT", name=f"oT{gi0}{hh}")
                    nc.vector.tensor_copy(oT[:], po_g[hh][:])
                    rs8 = gep.tile([8, P], F32, tag="rs8", name=f"rs8_{gi0}{hh}")
                    for c in range(8):
                        nc.gpsimd.dma_start(
                            rs8[c:c + 1, :], oT[64:65, c * P:(c + 1) * P]
                        )
                    rec8 = gep.tile([8, P], F32, tag="rec8", name=f"rec8_{gi0}{hh}")
                    nc.vector.reciprocal(rec8[:], rs8[:])
                    pending_norm.append((pr0, qh0, hh, oT, rec8))

            for gi, (pr, qh) in enumerate(groups):
                for tt in range(nt):
                    ebt = ebp.tile([P, 2, 1024], F16, tag="eb", name=f"eb{gi}{tt}")
                    nc.scalar.dma_start(ebt[:], d_eb[pr, qh, tt])
                    if gi == 0:
                        emit_v_tile(tt)
                        if tt in qk_slots:
                            emit_qk_tile(*qk_slots[tt])
                    if pending_norm and tt in (4, 6):
                        emit_norm_tail()
                    if gi == 2 and 7 <= tt < 15:
                        emit_fc_tile(tt - 7)
                    es0 = es0p.tile([P, 2, 1024], F16, tag="es0", name=f"es0_{gi}{tt}")
                    es1 = es1p.tile([P, 2, 1024], F16, tag="es1", name=f"es1_{gi}{tt}")
                    for hh in range(2):
                        sch = work.tile([P, 1024], F32, tag="w", name=f"sc{gi}{tt}{hh}")
                        for qc in range(2):
                            nc.tensor.matmul(
                                sch[:, qc * 512:(qc + 1) * 512],
                                lhsT=kT[hh * 64:(hh + 1) * 64, pr, tt * P:(tt + 1) * P],
                                rhs=qT[hh * 64:(hh + 1) * 64, pr,
                                       qh * 1024 + qc * 512: qh * 1024 + (qc + 1) * 512],
                                start=True,
                                stop=True,
                            )
                        nc.scalar.activation(es0[:, hh, :], sch[:], AF.Exp, scale=0.125)
                    nc.vector.tensor_tensor(es1[:], es0[:], ebt[:], OP.mult)
                    pend.append((gi, pr, tt, es1))
                    if len(pend) > AV_LAG:
                        emit_av()

            # tail: drain remaining AVs, last group's norms, qh=1 FC tiles
            while pend:
                emit_av()
            while pending_norm:
                emit_norm_tail()
            for qt in range(8, 16):
                emit_fc_tile(qt)

    if split_waits:
        _split_multi_waits(nc)
    return nc


# ---------------------------------------------------------------------------
# host-side packing
# ---------------------------------------------------------------------------


def _pack_x(x):
    """x: [S, D] fp32 -> [128, 8, S] fp16 (partition-major)."""
    return np.ascontiguousarray(
        x.T.reshape(8, P, x.shape[0]).transpose(1, 0, 2), dtype=np.float16
    )


def _pack_w_qk(W, h0):
    """W: [H, D, DH] -> [128(k), 2(pr), 8(dp), 128(j2)] fp16 for heads h0..h0+3."""
    Wp = np.zeros((D, 2, P), np.float32)
    for pr in range(2):
        for hh in range(2):
            Wp[:, pr, hh * 64:(hh + 1) * 64] = W[h0 + 2 * pr + hh]
    return np.ascontiguousarray(
        Wp.reshape(8, P, 2, P).transpose(1, 2, 0, 3), dtype=np.float16
    )


def _prep_core_inputs(c, query, key, value, ebfull, Wq, bq, Wk, bk, Wv, bv, Wfc):
    b = c // (NCORES // B)
    h0 = HPC * (c % (NCORES // B))
    f16 = np.float16

    bqk = np.zeros((P, 2, 2, 1), np.float32)
    for ty, bvec in ((0, bq), (1, bk)):
        for pr in range(2):
            for hh in range(2):
                bqk[hh * 64:(hh + 1) * 64, ty, pr, 0] = bvec[h0 + 2 * pr + hh]

    wv = np.zeros((D, HPC * 65), np.float32)
    for i in range(HPC):
        wv[:, i * 65:i * 65 + 64] = Wv[h0 + i]
    wv = np.ascontiguousarray(wv.reshape(8, P, HPC * 65).transpose(1, 0, 2), dtype=f16)
    # ones-column seed (bv itself is folded into the host-side output bias)
    bva = np.zeros((P, HPC * 65), np.float32)
    for i in range(HPC):
        bva[:, i * 65 + 64] = 1.0 / P

    # expb[pr, qh, tt, t, hh, q]; ebfull[b, h, q(query), t(key)] fp16
    eb = ebfull[b, h0:h0 + HPC]  # [4, S(q), S(t)]
    eb = eb.reshape(2, 2, 2, 1024, NT, P).transpose(0, 2, 4, 5, 1, 3)
    eb = np.ascontiguousarray(eb)

    wfc = np.zeros((P, 2, D), np.float32)
    for pr in range(2):
        for hh in range(2):
            h = h0 + 2 * pr + hh
            wfc[hh * 64:(hh + 1) * 64, pr, :] = Wfc[h * 64:(h + 1) * 64, :]

    sel = np.zeros((8, 512), np.float32)
    for c in range(8):
        sel[c, c * 64:(c + 1) * 64] = 1.0

    return {
        "xq": _pack_x(query[b]),
        "xk": _pack_x(key[b]),
        "xv": _pack_x(value[b]),
        "wq": _pack_w_qk(Wq, h0),
        "wk": _pack_w_qk(Wk, h0),
        "bqk": bqk,
        "wv": wv,
        "bv": bva.astype(f16),
        "eb": eb,
        "wfc": wfc.astype(f16),
        "sel": sel,
    }


def _install_ntff_hook():
    """The container's antenv stub lacks axon_hooks; synthesize it so
    trace=True can capture NTFF profiles via libaxon_pjrt.so ctypes calls."""
    import contextlib
    import ctypes
    import types

    import antenv

    if hasattr(antenv, "axon_hooks"):
        return
    so_path = "/opt/axon/libaxon_pjrt.so"
    try:
        lib = ctypes.CDLL(so_path)
    except OSError:
        return
    if not hasattr(lib, "axon_start_nrt_profile"):
        return
    lib.axon_start_nrt_profile.argtypes = [ctypes.POINTER(ctypes.c_int64), ctypes.c_size_t]
    lib.axon_start_nrt_profile.restype = ctypes.c_int64
    lib.axon_stop_nrt_profile.argtypes = [ctypes.c_char_p]
    lib.axon_stop_nrt_profile.restype = ctypes.c_int64

    @contextlib.contextmanager
    def _hook(output_dir, device_ids):
        import jax

        jax.devices()
        if device_ids:
            ids = (ctypes.c_int64 * len(device_ids))(*device_ids)
            rc = lib.axon_start_nrt_profile(ids, len(device_ids))
        else:
            rc = lib.axon_start_nrt_profile(None, 0)
        if rc != 0:
            raise RuntimeError(f"axon_start_nrt_profile rc={rc}")
        try:
            yield
        finally:
            n = lib.axon_stop_nrt_profile(str(output_dir).encode())
            print(f"profile: {n} file(s) written to {output_dir}", file=sys.stderr)

    mod = types.ModuleType("antenv.axon_hooks")
    mod._hook = _hook
    mod.get_axon_ntff_profile_hook = lambda: _hook
    mod.set_axon_ntff_profile_hook = lambda h: setattr(mod, "_hook", h)
    sys.modules["antenv.axon_hooks"] = mod
    antenv.axon_hooks = mod


def kernel(_trace=False, **inputs):
    from concourse.bass_utils import run_bass_kernel_spmd

    if _trace:
        _install_ntff_hook()
    if "nc" not in _cached:
        _cached["nc"] = _build_program()
    nc = _cached["nc"]

    args = {k: np.asarray(v) for k, v in inputs.items()}
    # exp(bias/8) once, in fp16 to halve host memory traffic
    ebfull = np.exp(
        args["relative_position_bias"].astype(np.float32) / 8.0
    ).astype(np.float16)
    in_maps = [
        _prep_core_inputs(
            c,
            args["query"], args["key"], args["value"], ebfull,
            args["Wq"], args["bq"], args["Wk"], args["bk"],
            args["Wv"], args["bv"], args["Wfc"],
        )
        for c in range(NCORES)
    ]

    res = run_bass_kernel_spmd(nc, in_maps, core_ids=list(range(NCORES)), trace=_trace)
    _cached["last_result"] = res

    # bv's contribution commutes through the softmax (weights sum to 1):
    # out += sum_h bv_h @ Wfc_h, a constant row, folded in here with bfc.
    hbias = args["bfc"].astype(np.float32).copy()
    for h in range(H):
        hbias += args["bv"][h].astype(np.float32) @ args["Wfc"][
            h * DH:(h + 1) * DH
        ].astype(np.float32)

    out = np.zeros((B, S, D), dtype=np.float32)
    cpb = NCORES // B
    for b in range(B):
        for i in range(cpb):
            out[b] += res.results[b * cpb + i]["out"].astype(np.float32)
        out[b] += hbias[None, :]
    return out


# revision 27
# speedup vs baseline: 1.2402x; 1.2402x over previous
"""Multi-head attention (per-head projections + relative position bias) on 8
Trainium2 NeuronCores.

Sharding: core c -> batch c//4, heads 4*(c%4) .. 4*(c%4)+4 (tensor parallel
over heads within a batch). Each core computes its 4 heads end-to-end plus the
partial output projection for those heads; the host sums the 4 partials per
batch and adds bfc.

Device-side design:
- bias folded multiplicatively: host ships expb = exp(bias/8) fp16; ACT does
  es0 = exp(scores/8) directly from PSUM (its fast port); DVE multiplies
  es = es0*expb in fp16 SBUF at 2x mode (instead of a 1x-mode fp32 PSUM add).
- one shared 4-bank PSUM work ring + 2x2-bank AV accumulators; projections, V,
  scores and FC all interleave through it so the PE never idles long enough to
  re-throttle the HAM clock gate.
- AV emission lags scores by 2 tiles so the in-order PE queue doesn't stall on
  exp latency.
- q/k biases preloaded into PSUM by K=1 ones-product matmuls; evacuations are
  plain DVE copies.
- softmax denominators ride the V ones-column (65th AV output row);
  reciprocal_approx_fast + a K=1 broadcast matmul builds the normalizer.
- FC contracts K=128 (head pair stacked on partitions via partition-shifted
  normalize writes); qh=0 FC interleaves into group 2, qh=1 FC is the tail.
- fp16 output partials; x loaded in halves so compute starts ~15us in.
"""

import sys

sys.path.insert(0, "/opt/trn_rl_repo")

import numpy as np

import concourse.bass as bass
import concourse.tile as tile_mod
from concourse import mybir

# ---------------------------------------------------------------------------
# This walrus build accepts only one sem-wait per CTRL/Drain instruction, so
# split the TileContext tail drain's waits onto individual single-wait nops.
# ---------------------------------------------------------------------------


def _patched_drain_and_barrier(self, tick_clock, wait_clock):
    nc = self.nc
    drain_inst = nc.sync.drain()
    wait_clock.add_sem_waits(
        drain_inst.ins, tile_mod.ScopedClock({None: tick_clock.global_clock})
    )
    si = drain_inst.ins.sync_info
    if si is not None and si.on_wait is not None and len(si.on_wait) > 1:
        waits = list(si.on_wait)
        si.on_wait = [waits[0]]
        for w in waits[1:]:
            n = nc.sync.nop()
            n.ins.sync_info = mybir.SyncInfo(on_wait=[w], on_update=[])

    nc.all_engine_barrier()
    assert self.sems is not None
    popped = nc._tile_sem_poison_stack.pop()
    assert popped is self._sem_poison
    nc.clear_and_free_semaphores(list(self.sems.allocated().values()))
    nc.all_engine_barrier()


tile_mod.TileContext._drain_and_barrier = _patched_drain_and_barrier

_split_ctr = [0]


def _split_multi_waits(nc):
    """Walrus here accepts a single sem-wait per instruction; hoist extra waits
    onto single-wait nops inserted just before, on the same engine."""
    for f in nc.m.functions:
        for bb in f.blocks:
            insts = bb.instructions
            out = []
            for inst in insts:
                si = inst.sync_info
                if si is not None and si.on_wait is not None and len(si.on_wait) > 1:
                    waits = list(si.on_wait)
                    for w in waits[:-1]:
                        _split_ctr[0] += 1
                        n = mybir.InstNoOp(name=f"splitw-{_split_ctr[0]}", ins=[], outs=[])
                        n.engine = inst.engine
                        n.sync_info = mybir.SyncInfo(on_wait=[w], on_update=[])
                        out.append(n)
                    inst.sync_info = mybir.SyncInfo(
                        on_wait=[waits[-1]], on_update=list(si.on_update or [])
                    )
                out.append(inst)
            if len(out) != len(insts):
                bb.instructions[:] = out


B, S, D, H, DH = 2, 2048, 1024, 16, 64
NCORES = 8
HPC = 4  # heads per core
P = 128
F16 = mybir.dt.float16
F32 = mybir.dt.float32
AF = mybir.ActivationFunctionType
OP = mybir.AluOpType

AV_LAG = 2
NT = 16  # t tiles of 128

_cached = {}


def _build_program(split_waits=True):
    nt, nqh = NT, 2
    Sl = nt * P
    Sq = nqh * 1024
    groups = ((0, 0), (1, 0), (0, 1), (1, 1))  # (pr, qh)

    nc = bass.Bass("TRN2", target_bir_lowering=False, debug=False)

    d_xq = nc.dram_tensor("xq", [P, 8, Sq], F16, kind="ExternalInput").ap()
    d_xk = nc.dram_tensor("xk", [P, 8, Sl], F16, kind="ExternalInput").ap()
    d_xv = nc.dram_tensor("xv", [P, 8, Sl], F16, kind="ExternalInput").ap()
    d_wq = nc.dram_tensor("wq", [P, 2, 8, P], F16, kind="ExternalInput").ap()
    d_wk = nc.dram_tensor("wk", [P, 2, 8, P], F16, kind="ExternalInput").ap()
    d_bqk = nc.dram_tensor("bqk", [P, 2, 2, 1], F32, kind="ExternalInput").ap()
    d_wv = nc.dram_tensor("wv", [P, 8, HPC * 65], F16, kind="ExternalInput").ap()
    # ones-column seed: bv128[k, j] = (j % 65 == 64) / 128, so a K=128 matmul
    # against all-ones weights initializes vv's denominator columns to 1.
    d_bv = nc.dram_tensor("bv", [P, HPC * 65], F16, kind="ExternalInput").ap()
    d_eb = nc.dram_tensor("eb", [2, nqh, nt, P, 2, 1024], F16, kind="ExternalInput").ap()
    d_wfc = nc.dram_tensor("wfc", [P, 2, D], F16, kind="ExternalInput").ap()
    d_sel = nc.dram_tensor("sel", [8, 512], F32, kind="ExternalInput").ap()
    d_out = nc.dram_tensor("out", [Sq, D], F16, kind="ExternalOutput").ap()

    with tile_mod.TileContext(nc) as tc:
        with tc.tile_pool(name="persist", bufs=1) as persist, \
             tc.tile_pool(name="work", bufs=2, space="PSUM") as work, \
             tc.tile_pool(name="ps_av", bufs=1, space="PSUM") as ps_av, \
             tc.tile_pool(name="ebp", bufs=5) as ebp, \
             tc.tile_pool(name="es0p", bufs=3) as es0p, \
             tc.tile_pool(name="es1p", bufs=3) as es1p, \
             tc.tile_pool(name="gep", bufs=2) as gep, \
             tc.tile_pool(name="fop", bufs=2) as fop:
            qT = persist.tile([P, 2, Sq], F16, tag="qT")  # [j2, pr, q]
            kT = persist.tile([P, 2, Sl], F16, tag="kT")  # [j2, pr, t]
            vv = persist.tile([P, nt, HPC * 65], F16, tag="vv")  # [t_in, tt, h*65+j]
            onorm = persist.tile([P, 2, Sq], F16, tag="onorm")  # [j2, pr, q]
            wq_sb = persist.tile([P, 2, 8, P], F16, tag="wq")
            wk_sb = persist.tile([P, 2, 8, P], F16, tag="wk")
            wv_sb = persist.tile([P, 8, HPC * 65], F16, tag="wv")
            wfc_sb = persist.tile([P, 2, D], F16, tag="wfc")
            xq_sb = persist.tile([P, 8, Sq], F16, tag="xq")
            xk_sb = persist.tile([P, 8, Sl], F16, tag="xk")
            xv_sb = persist.tile([P, 8, Sl], F16, tag="xv")
            bqk_sb = persist.tile([P, 2, 2, 1], F32, tag="bqk")
            bv_sb = persist.tile([P, HPC * 65], F16, tag="bv")
            ones_sb = persist.tile([P, P], F16, tag="ones")
            sel_sb = persist.tile([8, 512], F32, tag="sel")

            nc.vector.memset(ones_sb[:], 1.0)
            # --- DMA emission order matters: unblock k(pr0)/q(pr0,qh0) fast ---
            nc.sync.dma_start(sel_sb[:], d_sel)
            nc.sync.dma_start(bqk_sb[:], d_bqk)
            nc.sync.dma_start(bv_sb[:], d_bv)
            nc.sync.dma_start(wq_sb[:], d_wq)
            nc.sync.dma_start(wk_sb[:], d_wk)
            half = Sq // 2
            nc.sync.dma_start(xk_sb[:, :, 0:Sl // 2], d_xk[:, :, 0:Sl // 2])
            nc.sync.dma_start(xq_sb[:, :, 0:half], d_xq[:, :, 0:half])
            nc.sync.dma_start(xk_sb[:, :, Sl // 2:Sl], d_xk[:, :, Sl // 2:Sl])
            nc.sync.dma_start(wv_sb[:], d_wv)
            # xv in per-256 t chunks so V(tt) unblocks early
            for i in range(nt // 2):
                nc.sync.dma_start(
                    xv_sb[:, :, i * 256:(i + 1) * 256], d_xv[:, :, i * 256:(i + 1) * 256]
                )
            nc.sync.dma_start(xq_sb[:, :, half:Sq], d_xq[:, :, half:Sq])
            nc.sync.dma_start(wfc_sb[:], d_wfc)

            def emit_qk_tile(ty, pr, sh):
                """One projection tile -> qT/kT[:, pr, sh*1024:(sh+1)*1024]."""
                w_sb, x_sb, dst = (
                    (wq_sb, xq_sb, qT) if ty == 0 else (wk_sb, xk_sb, kT)
                )
                pw = work.tile([P, 1024], F32, tag="w", name=f"pw{ty}{pr}{sh}")
                for qc in range(2):
                    for dp in range(8):
                        nc.tensor.matmul(
                            pw[:, qc * 512:(qc + 1) * 512],
                            lhsT=w_sb[:, pr, dp, :],
                            rhs=x_sb[:, dp, sh * 1024 + qc * 512: sh * 1024 + (qc + 1) * 512],
                            start=(dp == 0),
                            stop=(dp == 7),
                        )
                nc.vector.tensor_scalar(
                    dst[:, pr, sh * 1024:(sh + 1) * 1024],
                    pw[:],
                    bqk_sb[:, ty, pr, :],
                    None,
                    OP.add,
                )

            def emit_v_tile(tt):
                pv = work.tile([P, 1024], F32, tag="w", name=f"pv{tt}")
                nc.tensor.matmul(
                    pv[:, 0:HPC * 65], lhsT=ones_sb[:], rhs=bv_sb[:],
                    start=True, stop=False,
                )
                for dp in range(8):
                    nc.tensor.matmul(
                        pv[:, 0:HPC * 65],
                        lhsT=xv_sb[:, dp, tt * P:(tt + 1) * P],
                        rhs=wv_sb[:, dp, :],
                        start=False,
                        stop=(dp == 7),
                    )
                nc.vector.tensor_copy(vv[:, tt, :], pv[:, 0:HPC * 65])

            def emit_fc_tile(qt):
                pf = work.tile([P, 1024], F32, tag="w", name=f"pf{qt}")
                for ec in range(2):
                    for i2 in range(2):
                        nc.tensor.matmul(
                            pf[:, ec * 512:(ec + 1) * 512],
                            lhsT=onorm[:, i2, qt * P:(qt + 1) * P],
                            rhs=wfc_sb[:, i2, ec * 512:(ec + 1) * 512],
                            start=(i2 == 0),
                            stop=(i2 == 1),
                        )
                fo = fop.tile([P, D], F16, tag="fo", name=f"fo{qt}")
                nc.vector.tensor_copy(fo[:], pf[:])
                nc.sync.dma_start(d_out[qt * P:(qt + 1) * P, :], fo[:])

            # remaining projection tiles interleaved into group 0's stream
            extra_qk = [(1, 1, 0), (1, 1, 1), (0, 1, 0), (0, 0, 1), (0, 1, 1)]
            qk_slots = {
                max(1, (i + 1) * nt // (len(extra_qk) + 1)): t
                for i, t in enumerate(extra_qk)
            }
            assert len(qk_slots) == len(extra_qk)

            # phase A head: what group 0 needs first (k01 last — its x half
            # arrives after xq's first half)
            emit_qk_tile(1, 0, 0)
            emit_qk_tile(0, 0, 0)
            emit_qk_tile(1, 0, 1)

            # normalize tails (rb matmul + onorm mult) deferred into the NEXT
            # group's instruction stream so the PE never waits on the
            # reciprocal chain at a group boundary.
            pending_norm = []

            def emit_norm_tail():
                pr0, qh0, hh, oT, rec8 = pending_norm.pop(0)
                rbt = work.tile([P, 1024], F32, tag="w", name=f"rb{pr0}{qh0}{hh}")
                for c in range(8):
                    nc.tensor.matmul(
                        rbt[0:64, c * P:(c + 1) * P],
                        lhsT=sel_sb[:, c * 64:(c + 1) * 64],
                        rhs=rec8[:],
                        start=True,
                        stop=True,
                    )
                nc.vector.tensor_tensor(
                    onorm[hh * 64:(hh + 1) * 64, pr0, qh0 * 1024:(qh0 + 1) * 1024],
                    oT[0:64, :],
                    rbt[0:64, :],
                    OP.mult,
                )

            # AV matmuls pend globally: a group's last AV batches interleave
            # with the NEXT group's score stream so the PE never drains dry
            # at a group boundary.
            pend = []
            po_by_gi = {}

            def emit_av():
                gi0, pr0, tt0, es1_t = pend.pop(0)
                if tt0 == 0:
                    # allocate accumulators only now, AFTER the previous
                    # group's final AV matmuls were emitted on these slots
                    po_by_gi[gi0] = [
                        ps_av.tile([65, 1024], F32, tag=f"po{hh}",
                                   name=f"po{gi0}{hh}")
                        for hh in range(2)
                    ]
                po_g = po_by_gi[gi0]
                for hh in range(2):
                    h = 2 * pr0 + hh
                    for qc in range(2):
                        nc.tensor.matmul(
                            po_g[hh][:, qc * 512:(qc + 1) * 512],
                            lhsT=vv[:, tt0, h * 65:(h + 1) * 65],
                            rhs=es1_t[:, hh, qc * 512:(qc + 1) * 512],
                            start=(tt0 == 0),
                            stop=(tt0 == nt - 1),
                        )
                if tt0 == nt - 1:
                    # group finished: kick off its normalization chain
                    emit_norm_head(po_g, pr0)

            norm_ctr = [0]

            def emit_norm_head(po_g, pr0):
                gi0 = norm_ctr[0]
                norm_ctr[0] += 1
                qh0 = groups[gi0][1]
                for hh in range(2):
                    oT = gep.tile([65, 1024], F32, tag="oT", name=f"oT{gi0}{hh}")
                    nc.vector.tensor_copy(oT[:], po_g[hh][:])
                    rs8 = gep.tile([8, P], F32, tag="rs8", name=f"rs8_{gi0}{hh}")
                    for c in range(8):
                        nc.gpsimd.dma_start(
                            rs8[c:c + 1, :], oT[64:65, c * P:(c + 1) * P]
                        )
                    rec8 = gep.tile([8, P], F32, tag="rec8", name=f"rec8_{gi0}{hh}")
                    nc.vector.reciprocal(rec8[:], rs8[:])
                    pending_norm.append((pr0, qh0, hh, oT, rec8))

            for gi, (pr, qh) in enumerate(groups):
                for tt in range(nt):
                    ebt = ebp.tile([P, 2, 1024], F16, tag="eb", name=f"eb{gi}{tt}")
                    nc.sync.dma_start(ebt[:], d_eb[pr, qh, tt])
                    if gi == 0:
                        emit_v_tile(tt)
                        if tt in qk_slots:
                            emit_qk_tile(*qk_slots[tt])
                    if pending_norm and tt in (4, 6):
                        emit_norm_tail()
                    if gi == 2 and 7 <= tt < 15:
                        emit_fc_tile(tt - 7)
                    es0 = es0p.tile([P, 2, 1024], F16, tag="es0", name=f"es0_{gi}{tt}")
                    es1 = es1p.tile([P, 2, 1024], F16, tag="es1", name=f"es1_{gi}{tt}")
                    for hh in range(2):
                        sch = work.tile([P, 1024], F32, tag="w", name=f"sc{gi}{tt}{hh}")
                        for qc in range(2):
                            nc.tensor.matmul(
                                sch[:, qc * 512:(qc + 1) * 512],
                                lhsT=kT[hh * 64:(hh + 1) * 64, pr, tt * P:(tt + 1) * P],
                                rhs=qT[hh * 64:(hh + 1) * 64, pr,
                                       qh * 1024 + qc * 512: qh * 1024 + (qc + 1) * 512],
                                start=True,
                                stop=True,
                            )
                        nc.scalar.activation(es0[:, hh, :], sch[:], AF.Exp, scale=0.125)
                    nc.vector.tensor_tensor(es1[:], es0[:], ebt[:], OP.mult)
                    pend.append((gi, pr, tt, es1))
                    if len(pend) > AV_LAG:
                        emit_av()

            # tail: drain remaining AVs, last group's norms, qh=1 FC tiles
            while pend:
                emit_av()
            while pending_norm:
                emit_norm_tail()
            for qt in range(8, 16):
                emit_fc_tile(qt)

    if split_waits:
        _split_multi_waits(nc)
    return nc


# ---------------------------------------------------------------------------
# host-side packing
# ---------------------------------------------------------------------------


def _pack_x(x):
    """x: [S, D] fp32 -> [128, 8, S] fp16 (partition-major)."""
    return np.ascontiguousarray(
        x.T.reshape(8, P, x.shape[0]).transpose(1, 0, 2), dtype=np.float16
    )


def _pack_w_qk(W, h0):
    """W: [H, D, DH] -> [128(k), 2(pr), 8(dp), 128(j2)] fp16 for heads h0..h0+3."""
    Wp = np.zeros((D, 2, P), np.float32)
    for pr in range(2):
        for hh in range(2):
            Wp[:, pr, hh * 64:(hh + 1) * 64] = W[h0 + 2 * pr + hh]
    return np.ascontiguousarray(
        Wp.reshape(8, P, 2, P).transpose(1, 2, 0, 3), dtype=np.float16
    )


def _prep_core_inputs(c, query, key, value, ebfull, Wq, bq, Wk, bk, Wv, bv, Wfc):
    b = c // (NCORES // B)
    h0 = HPC * (c % (NCORES // B))
    f16 = np.float16

    bqk = np.zeros((P, 2, 2, 1), np.float32)
    for ty, bvec in ((0, bq), (1, bk)):
        for pr in range(2):
            for hh in range(2):
                bqk[hh * 64:(hh + 1) * 64, ty, pr, 0] = bvec[h0 + 2 * pr + hh]

    wv = np.zeros((D, HPC * 65), np.float32)
    for i in range(HPC):
        wv[:, i * 65:i * 65 + 64] = Wv[h0 + i]
    wv = np.ascontiguousarray(wv.reshape(8, P, HPC * 65).transpose(1, 0, 2), dtype=f16)
    # ones-column seed (bv itself is folded into the host-side output bias)
    bva = np.zeros((P, HPC * 65), np.float32)
    for i in range(HPC):
        bva[:, i * 65 + 64] = 1.0 / P

    # expb[pr, qh, tt, t, hh, q]; ebfull[b, h, q(query), t(key)] fp16
    eb = ebfull[b, h0:h0 + HPC]  # [4, S(q), S(t)]
    eb = eb.reshape(2, 2, 2, 1024, NT, P).transpose(0, 2, 4, 5, 1, 3)
    eb = np.ascontiguousarray(eb)

    wfc = np.zeros((P, 2, D), np.float32)
    for pr in range(2):
        for hh in range(2):
            h = h0 + 2 * pr + hh
            wfc[hh * 64:(hh + 1) * 64, pr, :] = Wfc[h * 64:(h + 1) * 64, :]

    sel = np.zeros((8, 512), np.float32)
    for c in range(8):
        sel[c, c * 64:(c + 1) * 64] = 1.0

    return {
        "xq": _pack_x(query[b]),
        "xk": _pack_x(key[b]),
        "xv": _pack_x(value[b]),
        "wq": _pack_w_qk(Wq, h0),
        "wk": _pack_w_qk(Wk, h0),
        "bqk": bqk,
        "wv": wv,
        "bv": bva.astype(f16),
        "eb": eb,
        "wfc": wfc.astype(f16),
        "sel": sel,
    }


def _install_ntff_hook():
    """The container's antenv stub lacks axon_hooks; synthesize it so
    trace=True can capture NTFF profiles via libaxon_pjrt.so ctypes calls."""
    import contextlib
    import ctypes
    import types

    import antenv

    if hasattr(antenv, "axon_hooks"):
        return
    so_path = "/opt/axon/libaxon_pjrt.so"
    try:
        lib = ctypes.CDLL(so_path)
    except OSError:
        return
    if not hasattr(lib, "axon_start_nrt_profile"):
        return
    lib.axon_start_nrt_profile.argtypes = [ctypes.POINTER(ctypes.c_int64), ctypes.c_size_t]
    lib.axon_start_nrt_profile.restype = ctypes.c_int64
    lib.axon_stop_nrt_profile.argtypes = [ctypes.c_char_p]
    lib.axon_stop_nrt_profile.restype = ctypes.c_int64

    @contextlib.contextmanager
    def _hook(output_dir, device_ids):
        import jax

        jax.devices()
        if device_ids:
            ids = (ctypes.c_int64 * len(device_ids))(*device_ids)
            rc = lib.axon_start_nrt_profile(ids, len(device_ids))
        else:
            rc = lib.axon_start_nrt_profile(None, 0)
        if rc != 0:
            raise RuntimeError(f"axon_start_nrt_profile rc={rc}")
        try:
            yield
        finally:
            n = lib.axon_stop_nrt_profile(str(output_dir).encode())
            print(f"profile: {n} file(s) written to {output_dir}", file=sys.stderr)

    mod = types.ModuleType("antenv.axon_hooks")
    mod._hook = _hook
    mod.get_axon_ntff_profile_hook = lambda: _hook
    mod.set_axon_ntff_profile_hook = lambda h: setattr(mod, "_hook", h)
    sys.modules["antenv.axon_hooks"] = mod
    antenv.axon_hooks = mod


def kernel(_trace=False, **inputs):
    from concourse.bass_utils import run_bass_kernel_spmd

    if _trace:
        _install_ntff_hook()
    if "nc" not in _cached:
        _cached["nc"] = _build_program()
    nc = _cached["nc"]

    args = {k: np.asarray(v) for k, v in inputs.items()}
    # exp(bias/8) once, in fp16 to halve host memory traffic
    ebfull = np.exp(
        args["relative_position_bias"].astype(np.float32) / 8.0
    ).astype(np.float16)
    in_maps = [
        _prep_core_inputs(
            c,
            args["query"], args["key"], args["value"], ebfull,
            args["Wq"], args["bq"], args["Wk"], args["bk"],
            args["Wv"], args["bv"], args["Wfc"],
        )
        for c in range(NCORES)
    ]

    res = run_bass_kernel_spmd(nc, in_maps, core_ids=list(range(NCORES)), trace=_trace)
    _cached["last_result"] = res

    # bv's contribution commutes through the softmax (weights sum to 1):
    # out += sum_h bv_h @ Wfc_h, a constant row, folded in here with bfc.
    hbias = args["bfc"].astype(np.float32).copy()
    for h in range(H):
        hbias += args["bv"][h].astype(np.float32) @ args["Wfc"][
            h * DH:(h + 1) * DH
        ].astype(np.float32)

    out = np.zeros((B, S, D), dtype=np.float32)
    cpb = NCORES // B
    for b in range(B):
        for i in range(cpb):
            out[b] += res.results[b * cpb + i]["out"].astype(np.float32)
        out[b] += hbias[None, :]
    return out
